# revision 30
# baseline (speedup 1.0000x reference)
"""Causal multi-head self-attention on 8 Trainium2 NeuronCores.

Problem: B=4, T=2048, C=1024, H=16 heads (d=64), fp32 in/out.
    q/k/v = x @ W{q,k,v}.T + b{q,k,v}   (torch Linear convention)
    att   = softmax(causal_mask(q k^T / sqrt(d)))
    y     = (att v) @ Wp.T + bp

Sharding: batch (4) x head-group (2 groups of 8 heads) = 8 cores.
Each core computes a partial output-projection y_part for its batch and
head group; the host sums the two group partials per batch and adds bp.

Per-core kernel structure (all matmul operands bf16, f32 PSUM accum,
~3e-3 end-to-end scale-relative error vs the 2e-2 gate):
  - Host pre-transposes x and weights so every matmul operand has the
    contraction dim on SBUF partitions.
  - Attention computes S^T = K_h Q_h^T per head (keys on partitions);
    exp runs on ACT into bf16 probability tiles.
  - AV puts *queries* on the output partitions: out[128q, 65] accumulates
    P^T V over key chunks with moving dim 65 (64 d + denominator column
    from an all-ones V column), costing 65 PE rows per 128x128 key/query
    chunk instead of 512 -- half the AV PE time of the [65, 512] layout.
  - normalize_recip (GPSIMD) divides by the denominator column, then a
    PE transpose (identity matmul) returns each [128q, 64d] block to
    channel-major [64d, 128q] for the output projection.
  - Causality: chunks are skipped above the diagonal at 128 granularity
    in both S^T (query-sliced) and AV (query-chunk loop), and the
    diagonal 128x128 blocks are masked by one triangular multiply.
  - The i-block loop is interleaved with the QKV projection t-blocks
    (attention block b only needs projections of t < 512(b+1)), so the
    scalar engine's exp stream overlaps the PE projection matmuls.
    Head epilogues (copy/normalize/transpose) are deferred into the next
    pair's first segment so they hide behind the exp wait.
"""

import sys
import numpy as np

for _p in ("/opt/trn_rl_repo", "/root/.axon_site/_ro/trn_rl_repo"):
    if _p not in sys.path:
        sys.path.insert(0, _p)

B, T, C, H = 4, 2048, 1024, 16
G = 2                # head groups (cores per batch)
HPG = H // G         # heads per group/core = 8
D = C // H           # head dim = 64
CG = C // G          # channels per group = 512
TB = 256             # t-block width for the QKV projection phase
NTB = T // TB        # 8
IB = 512             # i-block (query block) width for attention
NIB = T // IB        # 4

LAST_RESULTS = None  # BassKernelResults of the most recent run (for test.py)


def _build_nc():
    from collections import deque
    import concourse.mybir as mybir
    import concourse.tile as tile
    from concourse import bacc
    from concourse.bass import ts, ds

    F32 = mybir.dt.float32
    BF16 = mybir.dt.bfloat16
    AF = mybir.ActivationFunctionType
    ALU = mybir.AluOpType

    nc = bacc.Bacc("TRN2")

    xT = nc.dram_tensor("xT", [C, T], BF16, kind="ExternalInput")
    wq = nc.dram_tensor("wq", [C, CG], BF16, kind="ExternalInput")
    wk = nc.dram_tensor("wk", [C, CG], BF16, kind="ExternalInput")
    wv = nc.dram_tensor("wv", [C, HPG * (D + 1)], BF16, kind="ExternalInput")
    wp = nc.dram_tensor("wp", [CG, C], BF16, kind="ExternalInput")
    bq = nc.dram_tensor("bq", [128, CG // 128], F32, kind="ExternalInput")
    bk = nc.dram_tensor("bk", [128, CG // 128], F32, kind="ExternalInput")
    bv = nc.dram_tensor("bv", [128, HPG * (D + 1)], F32, kind="ExternalInput")
    tri = nc.dram_tensor("tri", [128, 128], BF16, kind="ExternalInput")
    ident = nc.dram_tensor("ident", [128, 128], BF16, kind="ExternalInput")
    y = nc.dram_tensor("y", [T, C], BF16, kind="ExternalOutput")

    VW = HPG * (D + 1)  # 520 augmented V width
    NM = CG // 128      # 4 m-chunks of the per-group channel dim
    NC_ = C // 128      # 8 c-chunks of the contraction dim

    with tile.TileContext(nc) as tc:
        with tc.tile_pool(name="persist", bufs=1) as sbP, \
             tc.tile_pool(name="qtb", bufs=2) as sbQ, \
             tc.tile_pool(name="otb", bufs=4) as sbO, \
             tc.tile_pool(name="xb", bufs=2) as sbX, \
             tc.tile_pool(name="pt", bufs=3) as sbPt, \
             tc.tile_pool(name="yt", bufs=5) as sbY, \
             tc.tile_pool(name="nrm", bufs=3) as sbN, \
             tc.tile_pool(name="nrb", bufs=3) as sbNb, \
             tc.tile_pool(name="ps_mm", bufs=2, space="PSUM") as psMM, \
             tc.tile_pool(name="ps_s", bufs=2, space="PSUM") as psS, \
             tc.tile_pool(name="ps_o", bufs=2, space="PSUM") as psO:

            kt = sbP.tile([128, NM, T], BF16, tag="kt")
            vt = sbP.tile([128, T // 128, VW], BF16, tag="vt")
            bq_s = sbP.tile([128, NM], F32, tag="bq")
            bk_s = sbP.tile([128, NM], F32, tag="bk")
            bv_s = sbP.tile([128, VW], F32, tag="bv")
            tri_s = sbP.tile([128, 128], BF16, tag="tri")
            id_s = sbP.tile([128, 128], BF16, tag="id")

            wq_s = sbP.tile([128, NC_, CG], BF16, tag="wq")
            wk_s = sbP.tile([128, NC_, CG], BF16, tag="wk")
            wv_s = sbP.tile([128, NC_, VW], BF16, tag="wv")
            wp_s = sbP.tile([128, NM, C], BF16, tag="wp")
            xT_r = xT.rearrange("(o p) t -> p o t", p=128)
            wq_r = wq.rearrange("(o p) m -> p o m", p=128)

            def make_proj_units(tb, qtb):
                """QKV projection for t-columns [tb*TB, (tb+1)*TB) as a list
                of PE work units (each ~1-2 us) for interleaving."""
                state = {}

                def u_load():
                    xb = sbX.tile([128, NC_, TB], BF16, tag="xb", name="xb")
                    state["xb"] = xb
                    if tb == 0:
                        # startup critical path: spread the issue cost over
                        # three sequencers (565-667ns per dma_start) so the
                        # first matmul group (x halves + wq quarter 0) is
                        # in flight as early as possible. All other
                        # persistent loads are deferred behind x-block 0
                        # (see u_qk).
                        nc.sync.dma_start(xb[:, 0:NC_ // 2, :],
                                          xT_r[:, 0:NC_ // 2, ts(tb, TB)])
                        nc.scalar.dma_start(wq_s[:, :, 0:128],
                                            wq_r[:, :, 0:128])
                        nc.sync.dma_start(xb[:, NC_ // 2:NC_, :],
                                          xT_r[:, NC_ // 2:NC_, ts(tb, TB)])
                        nc.scalar.dma_start(bq_s[:], bq[:])
                        nc.scalar.dma_start(bk_s[:], bk[:])
                        for om in range(1, NM):
                            nc.sync.dma_start(
                                wq_s[:, :, ts(om, 128)], wq_r[:, :, ts(om, 128)])
                    else:
                        nc.sync.dma_start(xb[:], xT_r[:, :, ts(tb, TB)])

                def u_qk(w_s, b_s, dst, col, om):
                    def run():
                        xb = state["xb"]
                        pq = psMM.tile([128, TB], F32, tag="mm", name="pq")
                        for oc in range(NC_):
                            nc.tensor.matmul(
                                pq[:], w_s[:, oc, ts(om, 128)], xb[:, oc, :],
                                start=(oc == 0), stop=(oc == NC_ - 1))
                        nc.vector.tensor_tensor(
                            dst[:, om, ds(col, TB)], pq[:],
                            b_s[:, om, None].to_broadcast((128, TB)), ALU.add)
                        if tb == 0 and w_s is wq_s and om == NM - 1:
                            # defer remaining persistent loads behind x-block
                            # 0; wk in quarters (its first m-chunk is needed
                            # just ~2 units from now), bulk loads after
                            wk_r = wk.rearrange("(o p) m -> p o m", p=128)
                            for om2 in range(NM):
                                nc.sync.dma_start(
                                    wk_s[:, :, ts(om2, 128)],
                                    wk_r[:, :, ts(om2, 128)])
                            nc.scalar.dma_start(bv_s[:], bv[:])
                            nc.sync.dma_start(
                                wv_s[:], wv.rearrange("(o p) m -> p o m", p=128))
                            nc.sync.dma_start(
                                wp_s[:], wp.rearrange("(o p) n -> p o n", p=128))
                            nc.scalar.dma_start(tri_s[:], tri[:])
                            nc.scalar.dma_start(id_s[:], ident[:])
                    return run

                def u_v(tt):
                    def run():
                        xb = state["xb"]
                        jc = tb * (TB // 128) + tt
                        pv = psMM.tile([128, 512], F32, tag="mm", name="pv")
                        for oc in range(NC_):
                            nc.tensor.matmul(
                                pv[:], xb[:, oc, ts(tt, 128)], wv_s[:, oc, 0:512],
                                start=(oc == 0), stop=(oc == NC_ - 1))
                        pv1 = psMM.tile([128, VW - 512], F32, tag="mm", name="pv1")
                        for oc in range(NC_):
                            nc.tensor.matmul(
                                pv1[:], xb[:, oc, ts(tt, 128)],
                                wv_s[:, oc, 512:VW],
                                start=(oc == 0), stop=(oc == NC_ - 1))
                        nc.vector.tensor_tensor(
                            vt[:, jc, 0:512], pv[:], bv_s[:, 0:512], ALU.add)
                        nc.vector.tensor_tensor(
                            vt[:, jc, 512:VW], pv1[:], bv_s[:, 512:VW], ALU.add)
                    return run

                qcol = (tb % 2) * TB
                units = [u_load]
                for w_s, b_s, dst, col in (
                        (wq_s, bq_s, qtb, qcol), (wk_s, bk_s, kt, tb * TB)):
                    for om in range(NM):
                        units.append(u_qk(w_s, b_s, dst, col, om))
                v_units = [u_v(tt) for tt in range(TB // 128)]
                return units, v_units

            # deferred per-pair epilogue in two stages: stage A (psum copy +
            # normalize, DVE/Pool) is emitted inside the NEXT pair's first
            # segment, stage B (PE transposes + otb copy) inside its second
            # segment -- by then the normalize chain has certainly drained,
            # so the in-order PE stream never stalls on it
            pending = deque()

            def emit_pending(n=None):
                for _ in range(len(pending) if n is None else n):
                    if pending:
                        pending.popleft()()

            def make_epilogue(pair, heads, o_ps, otb):
                # both heads' normalized [128q, 64d] blocks land side by
                # side in one bf16 tile, so one [128,128] PE transpose
                # per qc yields the pair's full channel chunk
                st = {}

                def ep_a():
                    nrb = sbNb.tile([128, 4, 2 * D], BF16, tag="nrb",
                                    name="nrb")
                    st["nrb"] = nrb
                    for h in heads:
                        nrm = sbN.tile([128, 4, D + 1], F32, tag="nrm",
                                       name="nrm")
                        nc.vector.tensor_copy(nrm[:], o_ps[h][:, :, 0:D + 1])
                        for qc in range(4):
                            nc.gpsimd.normalize_recip(
                                nrb[:, qc, ds((h & 1) * D, D)],
                                nrm[:, qc, 0:D], nrm[:, qc, D:D + 1])

                def ep_b():
                    nrb = st["nrb"]
                    tp = psS.tile([128, 4, 128], BF16, tag="s", name="tp")
                    for qc in range(4):
                        nc.tensor.transpose(
                            tp[:, qc, :], nrb[:, qc, :], id_s[:])
                    nc.vector.tensor_copy(otb[:, pair, :], tp[:])
                return ep_a, ep_b

            def attention_block(b, qtb, otb, pool, per_slot):
                """Attention + softmax for queries [b*IB, (b+1)*IB).

                After each key-segment it emits up to `per_slot` queued PE
                work units from `pool`, so the in-order PE stream has
                projection work while waiting on exp."""
                # Bresenham pacing: distribute the pool evenly over all fire
                # points of this block
                total_points = 16 * (b + 1)
                n0 = len(pool)
                state = {"fired": 0, "emitted": 0}

                def fire(_n):
                    state["fired"] += 1
                    due = (state["fired"] * n0) // total_points
                    while state["emitted"] < due and pool:
                        pool.popleft()()
                        state["emitted"] += 1

                nch = 4 * (b + 1)  # causal j-chunks
                for pair in range(HPG // 2):
                    heads = (2 * pair, 2 * pair + 1)
                    o_ps = {}
                    for h in heads:
                        # exactly one 2KB PSUM bank: the 4 qc accumulation
                        # groups interleave inside it, so the bank must not
                        # be shared (start=True clears has_written bits for
                        # the WHOLE bank)
                        o_ps[h] = psO.tile(
                            [128, 4, 128], F32, tag="o", name=f"o{h & 1}")
                    for seg in range(nch // 2):
                        sps, pts, loss = {}, {}, {}
                        for h in heads:
                            po, oh = (h & 1) * 64, h >> 1
                            sp = psS.tile([128, 2, 512], F32, tag="s", name="sp")
                            sps[h] = sp
                            los = {}
                            for u in (0, 1):
                                jc = 2 * seg + u
                                k = jc - 4 * b  # >=0 on diagonal chunks
                                los[u] = 0 if k <= 0 else 128 * k
                                # the two heads of a pair sit in disjoint
                                # 64-row groups of the PE array, so their
                                # K=64 matmuls run concurrently
                                nc.tensor.matmul(
                                    sp[:, u, los[u]:512],
                                    kt[po:po + 64, oh, ts(jc, 128)],
                                    qtb[po:po + 64, oh, los[u]:512],
                                    start=True, stop=True,
                                    tile_position=(po, 0))
                            loss[h] = los
                        for h in heads:
                            sp, los = sps[h], loss[h]
                            pt = sbPt.tile([128, 2, 512], BF16, tag="p",
                                           name="pt")
                            pts[h] = pt
                            if 2 * seg >= 4 * b:  # diagonal segment
                                # one exp covers both chunks from the smaller
                                # column offset; the extra columns of the
                                # second chunk are never read by its AV
                                lo = min(los[0], los[1])
                                nc.scalar.activation(
                                    pt[:, :, lo:512], sp[:, :, lo:512],
                                    AF.Exp, scale=0.125)
                                for u in (0, 1):
                                    k = 2 * seg + u - 4 * b  # 0..3
                                    nc.vector.tensor_tensor(
                                        pt[:, u, los[u]:los[u] + 128],
                                        pt[:, u, los[u]:los[u] + 128],
                                        tri_s, ALU.mult)
                            else:
                                nc.scalar.activation(
                                    pt[:, :, :], sp[:, :, :], AF.Exp,
                                    scale=0.125)
                        if seg <= 1:
                            emit_pending(1)
                        fire(per_slot)
                        for h in heads:
                            pt = pts[h]
                            for u in (0, 1):
                                jc = 2 * seg + u
                                k = jc - 4 * b
                                for qc in range(max(0, k), 4):
                                    # start=True only on the bank's first
                                    # matmul: it clears the whole bank's
                                    # has_written bits, so each qc region's
                                    # first write is an overwrite (bit
                                    # clear) and later writes accumulate
                                    nc.tensor.matmul(
                                        o_ps[h][:, qc, 0:D + 1],
                                        pt[:, u, ts(qc, 128)],
                                        vt[:, jc, ds((D + 1) * h, D + 1)],
                                        start=(jc == 0 and qc == 0),
                                        stop=(jc == 4 * b + qc),
                                        skip_group_check=True)
                        fire(per_slot)
                    pending.extend(make_epilogue(pair, heads, o_ps, otb))

            def make_yproj_units(b, otb, split_last=False):
                def u_y(it, nb, halves=1):
                    def run():
                        ic = (IB // 128) * b + it
                        py = psMM.tile([128, 512], F32, tag="mm", name="py")
                        for om in range(NM):
                            nc.tensor.matmul(
                                py[:], otb[:, om, ts(it, 128)],
                                wp_s[:, om, ts(nb, 512)],
                                start=(om == 0), stop=(om == NM - 1))
                        yt = sbY.tile([128, 512], BF16, tag="yt", name="yt")
                        # split the copy+DMA so the final drain is short
                        hw_ = 512 // halves
                        for hh in range(halves):
                            nc.vector.tensor_copy(
                                yt[:, ds(hh * hw_, hw_)],
                                py[:, ds(hh * hw_, hw_)])
                            nc.sync.dma_start(
                                y[ds(128 * ic, 128),
                                  ds(512 * nb + hh * hw_, hw_)],
                                yt[:, ds(hh * hw_, hw_)])
                    return run

                return [u_y(it, nb,
                            2 if (split_last and it == IB // 128 - 1
                                  and nb == C // 512 - 1) else 1)
                        for it in range(IB // 128) for nb in range(C // 512)]

            # ---- interleaved schedule ------------------------------------
            # proj(2b+2, 2b+3) and deferred yproj units are drip-fed into
            # attention(b)'s seg loop; everything left over flushes at the
            # block boundary (proj must finish before attention(b+1) reads it)
            import math as _math

            pool = deque()
            q_tiles = {}
            q_tiles[0] = sbQ.tile([128, NM, IB], BF16, tag="qtb", name="qt0")
            for tb in (0, 1):
                us, vs = make_proj_units(tb, q_tiles[0])
                for u in us + vs:
                    u()
            o_tiles = {}
            for b in range(NIB):
                o_tiles[b] = sbO.tile(
                    [128, NM, IB], BF16, tag="otb", name=f"ot{b}")
                if b < NIB - 1:
                    q_tiles[b + 1] = sbQ.tile(
                        [128, NM, IB], BF16, tag="qtb", name=f"qt{(b + 1) & 1}")
                    usA, vsA = make_proj_units(2 * b + 2, q_tiles[b + 1])
                    usB, vsB = make_proj_units(2 * b + 3, q_tiles[b + 1])
                    if b < 2:
                        pool.extend(usA + vsA + usB + vsB)
                    else:
                        pool.extend(usA + vsA + usB + vsB)
                        deferred_v = []
                # weight the yproj fill toward late blocks, which are
                # exp-(ACT-)bound and need PE fill work: block 3 has ~30us
                # of PE slack, blocks 1-2 are mostly covered by proj units
                held = []
                if b == 2:
                    yp0 = make_yproj_units(0, o_tiles[0])
                    pool.extend(yp0[:8])
                    leftover_yp = yp0[8:]
                elif b == 3:
                    pool.extend(deferred_v)
                    yp = leftover_yp + make_yproj_units(1, o_tiles[1]) \
                        + make_yproj_units(2, o_tiles[2])
                    held = yp[-3:]  # cover the last pair's epilogue latency
                    pool.extend(yp[:-3])
                slots = 16 * (b + 1)
                per_slot = max(1, _math.ceil(len(pool) / slots))
                attention_block(b, q_tiles[b], o_tiles[b], pool, per_slot)
                while pool:  # flush before the next block depends on it
                    pool.popleft()()
            for u in held:  # PE work while the last pair's epilogue drains
                u()
            emit_pending()  # last pair of block 3
            for u in make_yproj_units(NIB - 1, o_tiles[NIB - 1],
                                      split_last=True):
                u()

    nc.finalize()
    return nc


def _prep_core_inputs(x, Wq, bq, Wk, bk, Wv, bv, Wp, core):
    import ml_dtypes
    BF = ml_dtypes.bfloat16
    b, g = core // G, core % G
    rows = slice(CG * g, CG * (g + 1))
    xT = np.ascontiguousarray(x[b].T).astype(BF)
    wqT = np.ascontiguousarray(Wq[rows, :].T).astype(BF)
    wkT = np.ascontiguousarray(Wk[rows, :].T).astype(BF)
    wvT = np.zeros((C, HPG * (D + 1)), dtype=np.float32)
    bv_aug = np.zeros(HPG * (D + 1), dtype=np.float32)
    for hl in range(HPG):
        cols = slice((D + 1) * hl, (D + 1) * hl + D)
        wvT[:, cols] = Wv[CG * g + D * hl: CG * g + D * (hl + 1), :].T
        bv_aug[cols] = bv[CG * g + D * hl: CG * g + D * (hl + 1)]
        bv_aug[(D + 1) * hl + D] = 1.0
    wpT = np.ascontiguousarray(Wp[:, rows].T).astype(BF)
    bqT = np.ascontiguousarray(bq[rows].reshape(CG // 128, 128).T)
    bkT = np.ascontiguousarray(bk[rows].reshape(CG // 128, 128).T)
    bvR = np.ascontiguousarray(np.tile(bv_aug[None, :], (128, 1)))
    tri = np.triu(np.ones((128, 128), dtype=np.float32)).astype(BF)
    ident = np.eye(128, dtype=np.float32).astype(BF)
    return {
        "xT": xT, "wq": wqT, "wk": wkT, "wv": wvT.astype(BF),
        "wp": wpT, "bq": bqT, "bk": bkT, "bv": bvR,
        "tri": tri, "ident": ident,
    }


def kernel(x, Wq, bq, Wk, bk, Wv, bv, Wp, bp, _trace=False):
    global LAST_RESULTS
    from concourse.bass_utils import run_bass_kernel_spmd

    x = np.asarray(x, dtype=np.float32)
    Wq, bq = np.asarray(Wq, np.float32), np.asarray(bq, np.float32)
    Wk, bk = np.asarray(Wk, np.float32), np.asarray(bk, np.float32)
    Wv, bv = np.asarray(Wv, np.float32), np.asarray(bv, np.float32)
    Wp, bp = np.asarray(Wp, np.float32), np.asarray(bp, np.float32)

    nc = _build_nc()
    in_maps = [
        _prep_core_inputs(x, Wq, bq, Wk, bk, Wv, bv, Wp, core)
        for core in range(B * G)
    ]
    res = run_bass_kernel_spmd(nc, in_maps, core_ids=list(range(8)), trace=_trace)
    LAST_RESULTS = res

    out = np.empty((B, T, C), dtype=np.float32)
    for b in range(B):
        out[b] = (np.asarray(res.results[G * b]["y"], dtype=np.float32)
                  + np.asarray(res.results[G * b + 1]["y"], dtype=np.float32)
                  + bp[None, :])
    return out


# revision 32
# speedup vs baseline: 1.0146x; 1.0146x over previous
"""Causal multi-head self-attention on 8 Trainium2 NeuronCores.

Problem: B=4, T=2048, C=1024, H=16 heads (d=64), fp32 in/out.
    q/k/v = x @ W{q,k,v}.T + b{q,k,v}   (torch Linear convention)
    att   = softmax(causal_mask(q k^T / sqrt(d)))
    y     = (att v) @ Wp.T + bp

Sharding: batch (4) x head-group (2 groups of 8 heads) = 8 cores.
Each core computes a partial output-projection y_part for its batch and
head group; the host sums the two group partials per batch and adds bp.

Per-core kernel structure (all matmul operands bf16, f32 PSUM accum,
~3e-3 end-to-end scale-relative error vs the 2e-2 gate):
  - Host pre-transposes x and weights so every matmul operand has the
    contraction dim on SBUF partitions.
  - Attention computes S^T = K_h Q_h^T per head (keys on partitions);
    exp runs on ACT into bf16 probability tiles.
  - AV puts *queries* on the output partitions: out[128q, 65] accumulates
    P^T V over key chunks with moving dim 65 (64 d + denominator column
    from an all-ones V column), costing 65 PE rows per 128x128 key/query
    chunk instead of 512 -- half the AV PE time of the [65, 512] layout.
  - normalize_recip (GPSIMD) divides by the denominator column, then a
    PE transpose (identity matmul) returns each [128q, 64d] block to
    channel-major [64d, 128q] for the output projection.
  - Causality: chunks are skipped above the diagonal at 128 granularity
    in both S^T (query-sliced) and AV (query-chunk loop), and the
    diagonal 128x128 blocks are masked by one triangular multiply.
  - The i-block loop is interleaved with the QKV projection t-blocks
    (attention block b only needs projections of t < 512(b+1)), so the
    scalar engine's exp stream overlaps the PE projection matmuls.
    Head epilogues (copy/normalize/transpose) are deferred into the next
    pair's first segment so they hide behind the exp wait.
"""

import sys
import numpy as np

for _p in ("/opt/trn_rl_repo", "/root/.axon_site/_ro/trn_rl_repo"):
    if _p not in sys.path:
        sys.path.insert(0, _p)

B, T, C, H = 4, 2048, 1024, 16
G = 2                # head groups (cores per batch)
HPG = H // G         # heads per group/core = 8
D = C // H           # head dim = 64
CG = C // G          # channels per group = 512
TB = 256             # t-block width for the QKV projection phase
NTB = T // TB        # 8
IB = 512             # i-block (query block) width for attention
NIB = T // IB        # 4

LAST_RESULTS = None  # BassKernelResults of the most recent run (for test.py)


def _build_nc():
    from collections import deque
    import concourse.mybir as mybir
    import concourse.tile as tile
    from concourse import bacc
    from concourse.bass import ts, ds

    F32 = mybir.dt.float32
    BF16 = mybir.dt.bfloat16
    AF = mybir.ActivationFunctionType
    ALU = mybir.AluOpType

    nc = bacc.Bacc("TRN2")

    xT = nc.dram_tensor("xT", [C, T], BF16, kind="ExternalInput")
    wq = nc.dram_tensor("wq", [C, CG], BF16, kind="ExternalInput")
    wk = nc.dram_tensor("wk", [C, CG], BF16, kind="ExternalInput")
    wv = nc.dram_tensor("wv", [C, HPG * (D + 1)], BF16, kind="ExternalInput")
    wp = nc.dram_tensor("wp", [CG, C], BF16, kind="ExternalInput")
    bq = nc.dram_tensor("bq", [128, CG // 128], F32, kind="ExternalInput")
    bk = nc.dram_tensor("bk", [128, CG // 128], F32, kind="ExternalInput")
    bv = nc.dram_tensor("bv", [128, HPG * (D + 1)], F32, kind="ExternalInput")
    tri = nc.dram_tensor("tri", [128, 128], BF16, kind="ExternalInput")
    ident = nc.dram_tensor("ident", [128, 128], BF16, kind="ExternalInput")
    y = nc.dram_tensor("y", [T, C], BF16, kind="ExternalOutput")

    VW = HPG * (D + 1)  # 520 augmented V width
    NM = CG // 128      # 4 m-chunks of the per-group channel dim
    NC_ = C // 128      # 8 c-chunks of the contraction dim

    with tile.TileContext(nc) as tc:
        with tc.tile_pool(name="persist", bufs=1) as sbP, \
             tc.tile_pool(name="qtb", bufs=2) as sbQ, \
             tc.tile_pool(name="otb", bufs=4) as sbO, \
             tc.tile_pool(name="xb", bufs=2) as sbX, \
             tc.tile_pool(name="pt", bufs=3) as sbPt, \
             tc.tile_pool(name="yt", bufs=5) as sbY, \
             tc.tile_pool(name="nrm", bufs=3) as sbN, \
             tc.tile_pool(name="nrb", bufs=3) as sbNb, \
             tc.tile_pool(name="ps_mm", bufs=2, space="PSUM") as psMM, \
             tc.tile_pool(name="ps_s", bufs=2, space="PSUM") as psS, \
             tc.tile_pool(name="ps_o", bufs=2, space="PSUM") as psO:

            kt = sbP.tile([128, NM, T], BF16, tag="kt")
            vt = sbP.tile([128, T // 128, VW], BF16, tag="vt")
            bq_s = sbP.tile([128, NM], F32, tag="bq")
            bk_s = sbP.tile([128, NM], F32, tag="bk")
            bv_s = sbP.tile([128, VW], F32, tag="bv")
            tri_s = sbP.tile([128, 128], BF16, tag="tri")
            id_s = sbP.tile([128, 128], BF16, tag="id")

            wq_s = sbP.tile([128, NC_, CG], BF16, tag="wq")
            wk_s = sbP.tile([128, NC_, CG], BF16, tag="wk")
            wv_s = sbP.tile([128, NC_, VW], BF16, tag="wv")
            wp_s = sbP.tile([128, NM, C], BF16, tag="wp")
            xT_r = xT.rearrange("(o p) t -> p o t", p=128)
            wq_r = wq.rearrange("(o p) m -> p o m", p=128)

            def make_proj_units(tb, qtb):
                """QKV projection for t-columns [tb*TB, (tb+1)*TB) as a list
                of PE work units (each ~1-2 us) for interleaving."""
                state = {}

                def u_load():
                    xb = sbX.tile([128, NC_, TB], BF16, tag="xb", name="xb")
                    state["xb"] = xb
                    if tb == 0:
                        # startup critical path: spread the issue cost over
                        # three sequencers (565-667ns per dma_start) so the
                        # first matmul group (x halves + wq quarter 0) is
                        # in flight as early as possible. All other
                        # persistent loads are deferred behind x-block 0
                        # (see u_qk).
                        nc.sync.dma_start(xb[:, 0:NC_ // 2, :],
                                          xT_r[:, 0:NC_ // 2, ts(tb, TB)])
                        nc.scalar.dma_start(wq_s[:, :, 0:128],
                                            wq_r[:, :, 0:128])
                        nc.sync.dma_start(xb[:, NC_ // 2:NC_, :],
                                          xT_r[:, NC_ // 2:NC_, ts(tb, TB)])
                        nc.scalar.dma_start(bq_s[:], bq[:])
                        nc.scalar.dma_start(bk_s[:], bk[:])
                        for om in range(1, NM):
                            nc.sync.dma_start(
                                wq_s[:, :, ts(om, 128)], wq_r[:, :, ts(om, 128)])
                    else:
                        nc.sync.dma_start(xb[:], xT_r[:, :, ts(tb, TB)])

                def u_qk(w_s, b_s, dst, col, om):
                    def run():
                        xb = state["xb"]
                        pq = psMM.tile([128, TB], F32, tag="mm", name="pq")
                        for oc in range(NC_):
                            nc.tensor.matmul(
                                pq[:], w_s[:, oc, ts(om, 128)], xb[:, oc, :],
                                start=(oc == 0), stop=(oc == NC_ - 1))
                        nc.vector.tensor_tensor(
                            dst[:, om, ds(col, TB)], pq[:],
                            b_s[:, om, None].to_broadcast((128, TB)), ALU.add)
                        if tb == 0 and w_s is wq_s and om == NM - 1:
                            # defer remaining persistent loads behind x-block
                            # 0; wk in quarters (its first m-chunk is needed
                            # just ~2 units from now), bulk loads after
                            wk_r = wk.rearrange("(o p) m -> p o m", p=128)
                            for om2 in range(NM):
                                nc.sync.dma_start(
                                    wk_s[:, :, ts(om2, 128)],
                                    wk_r[:, :, ts(om2, 128)])
                            nc.scalar.dma_start(bv_s[:], bv[:])
                            nc.sync.dma_start(
                                wv_s[:], wv.rearrange("(o p) m -> p o m", p=128))
                            nc.sync.dma_start(
                                wp_s[:], wp.rearrange("(o p) n -> p o n", p=128))
                            nc.scalar.dma_start(tri_s[:], tri[:])
                            nc.scalar.dma_start(id_s[:], ident[:])
                    return run

                def u_v(tt):
                    def run():
                        xb = state["xb"]
                        jc = tb * (TB // 128) + tt
                        pv = psMM.tile([128, 512], F32, tag="mm", name="pv")
                        for oc in range(NC_):
                            nc.tensor.matmul(
                                pv[:], xb[:, oc, ts(tt, 128)], wv_s[:, oc, 0:512],
                                start=(oc == 0), stop=(oc == NC_ - 1))
                        pv1 = psMM.tile([128, VW - 512], F32, tag="mm", name="pv1")
                        for oc in range(NC_):
                            nc.tensor.matmul(
                                pv1[:], xb[:, oc, ts(tt, 128)],
                                wv_s[:, oc, 512:VW],
                                start=(oc == 0), stop=(oc == NC_ - 1))
                        nc.vector.tensor_tensor(
                            vt[:, jc, 0:512], pv[:], bv_s[:, 0:512], ALU.add)
                        nc.vector.tensor_tensor(
                            vt[:, jc, 512:VW], pv1[:], bv_s[:, 512:VW], ALU.add)
                    return run

                qcol = (tb % 2) * TB
                units = [u_load]
                for w_s, b_s, dst, col in (
                        (wq_s, bq_s, qtb, qcol), (wk_s, bk_s, kt, tb * TB)):
                    for om in range(NM):
                        units.append(u_qk(w_s, b_s, dst, col, om))
                v_units = [u_v(tt) for tt in range(TB // 128)]
                return units, v_units

            # deferred per-pair epilogue in two stages: stage A (psum copy +
            # normalize, DVE/Pool) is emitted inside the NEXT pair's first
            # segment, stage B (PE transposes + otb copy) inside its second
            # segment -- by then the normalize chain has certainly drained,
            # so the in-order PE stream never stalls on it
            pending = deque()

            def emit_pending(n=None):
                for _ in range(len(pending) if n is None else n):
                    if pending:
                        pending.popleft()()

            def make_epilogue(pair, heads, o_ps, otb):
                # both heads' normalized [128q, 64d] blocks land side by
                # side in one bf16 tile, so one [128,128] PE transpose
                # per qc yields the pair's full channel chunk
                st = {}

                def ep_a():
                    nrb = sbNb.tile([128, 4, 2 * D], BF16, tag="nrb",
                                    name="nrb")
                    st["nrb"] = nrb
                    for h in heads:
                        nrm = sbN.tile([128, 4, D + 1], F32, tag="nrm",
                                       name="nrm")
                        nc.vector.tensor_copy(nrm[:], o_ps[h][:, :, 0:D + 1])
                        for qc in range(4):
                            nc.gpsimd.normalize_recip(
                                nrb[:, qc, ds((h & 1) * D, D)],
                                nrm[:, qc, 0:D], nrm[:, qc, D:D + 1])

                def ep_b():
                    nrb = st["nrb"]
                    tp = psS.tile([128, 4, 128], BF16, tag="s", name="tp")
                    for qc in range(4):
                        nc.tensor.transpose(
                            tp[:, qc, :], nrb[:, qc, :], id_s[:])
                    nc.vector.tensor_copy(otb[:, pair, :], tp[:])
                return ep_a, ep_b

            def attention_block(b, qtb, otb, pool, per_slot):
                """Attention + softmax for queries [b*IB, (b+1)*IB).

                After each key-segment it emits up to `per_slot` queued PE
                work units from `pool`, so the in-order PE stream has
                projection work while waiting on exp."""
                # Bresenham pacing: distribute the pool evenly over all fire
                # points of this block
                total_points = 16 * (b + 1)
                n0 = len(pool)
                state = {"fired": 0, "emitted": 0}

                def fire(_n):
                    state["fired"] += 1
                    due = (state["fired"] * n0) // total_points
                    while state["emitted"] < due and pool:
                        pool.popleft()()
                        state["emitted"] += 1

                nch = 4 * (b + 1)  # causal j-chunks
                for pair in range(HPG // 2):
                    heads = (2 * pair, 2 * pair + 1)
                    o_ps = {}
                    for h in heads:
                        # exactly one 2KB PSUM bank: the 4 qc accumulation
                        # groups interleave inside it, so the bank must not
                        # be shared (start=True clears has_written bits for
                        # the WHOLE bank)
                        o_ps[h] = psO.tile(
                            [128, 4, 128], F32, tag="o", name=f"o{h & 1}")
                    for seg in range(nch // 2):
                        sps, pts, loss = {}, {}, {}
                        for h in heads:
                            po, oh = (h & 1) * 64, h >> 1
                            sp = psS.tile([128, 2, 512], F32, tag="s", name="sp")
                            sps[h] = sp
                            los = {}
                            for u in (0, 1):
                                jc = 2 * seg + u
                                k = jc - 4 * b  # >=0 on diagonal chunks
                                los[u] = 0 if k <= 0 else 128 * k
                                # the two heads of a pair sit in disjoint
                                # 64-row groups of the PE array, so their
                                # K=64 matmuls run concurrently
                                nc.tensor.matmul(
                                    sp[:, u, los[u]:512],
                                    kt[po:po + 64, oh, ts(jc, 128)],
                                    qtb[po:po + 64, oh, los[u]:512],
                                    start=True, stop=True,
                                    tile_position=(po, 0))
                            loss[h] = los
                        for h in heads:
                            sp, los = sps[h], loss[h]
                            pt = sbPt.tile([128, 2, 512], BF16, tag="p",
                                           name="pt")
                            pts[h] = pt
                            if 2 * seg >= 4 * b:  # diagonal segment
                                # one exp covers both chunks from the smaller
                                # column offset; the extra columns of the
                                # second chunk are never read by its AV
                                lo = min(los[0], los[1])
                                nc.scalar.activation(
                                    pt[:, :, lo:512], sp[:, :, lo:512],
                                    AF.Exp, scale=0.125)
                                for u in (0, 1):
                                    k = 2 * seg + u - 4 * b  # 0..3
                                    nc.vector.tensor_tensor(
                                        pt[:, u, los[u]:los[u] + 128],
                                        pt[:, u, los[u]:los[u] + 128],
                                        tri_s, ALU.mult)
                            else:
                                nc.scalar.activation(
                                    pt[:, :, :], sp[:, :, :], AF.Exp,
                                    scale=0.125)
                        if seg <= 1:
                            emit_pending(1)
                        fire(per_slot)
                        for h in heads:
                            pt = pts[h]
                            for u in (0, 1):
                                jc = 2 * seg + u
                                k = jc - 4 * b
                                for qc in range(max(0, k), 4):
                                    # start=True only on the bank's first
                                    # matmul: it clears the whole bank's
                                    # has_written bits, so each qc region's
                                    # first write is an overwrite (bit
                                    # clear) and later writes accumulate
                                    nc.tensor.matmul(
                                        o_ps[h][:, qc, 0:D + 1],
                                        pt[:, u, ts(qc, 128)],
                                        vt[:, jc, ds((D + 1) * h, D + 1)],
                                        start=(jc == 0 and qc == 0),
                                        stop=(jc == 4 * b + qc),
                                        skip_group_check=True)
                        fire(per_slot)
                    pending.extend(make_epilogue(pair, heads, o_ps, otb))

            def make_yproj_units(b, otb, split_last=False):
                def u_y(it, nb, halves=1):
                    def run():
                        ic = (IB // 128) * b + it
                        py = psMM.tile([128, 512], F32, tag="mm", name="py")
                        for om in range(NM):
                            nc.tensor.matmul(
                                py[:], otb[:, om, ts(it, 128)],
                                wp_s[:, om, ts(nb, 512)],
                                start=(om == 0), stop=(om == NM - 1))
                        yt = sbY.tile([128, 512], BF16, tag="yt", name="yt")
                        # split the copy+DMA so the final drain is short
                        hw_ = 512 // halves
                        for hh in range(halves):
                            nc.vector.tensor_copy(
                                yt[:, ds(hh * hw_, hw_)],
                                py[:, ds(hh * hw_, hw_)])
                            nc.sync.dma_start(
                                y[ds(128 * ic, 128),
                                  ds(512 * nb + hh * hw_, hw_)],
                                yt[:, ds(hh * hw_, hw_)])
                    return run

                return [u_y(it, nb,
                            2 if (split_last and it == IB // 128 - 1
                                  and nb == C // 512 - 1) else 1)
                        for it in range(IB // 128) for nb in range(C // 512)]

            # ---- interleaved schedule ------------------------------------
            # proj(2b+2, 2b+3) and deferred yproj units are drip-fed into
            # attention(b)'s seg loop; everything left over flushes at the
            # block boundary (proj must finish before attention(b+1) reads it)
            import math as _math

            pool = deque()
            q_tiles = {}
            q_tiles[0] = sbQ.tile([128, NM, IB], BF16, tag="qtb", name="qt0")
            for tb in (0, 1):
                us, vs = make_proj_units(tb, q_tiles[0])
                for u in us + vs:
                    u()
            o_tiles = {}
            for b in range(NIB):
                o_tiles[b] = sbO.tile(
                    [128, NM, IB], BF16, tag="otb", name=f"ot{b}")
                if b < NIB - 1:
                    q_tiles[b + 1] = sbQ.tile(
                        [128, NM, IB], BF16, tag="qtb", name=f"qt{(b + 1) & 1}")
                    usA, vsA = make_proj_units(2 * b + 2, q_tiles[b + 1])
                    usB, vsB = make_proj_units(2 * b + 3, q_tiles[b + 1])
                    if b < 2:
                        pool.extend(usA + vsA + usB + vsB)
                    else:
                        # blocks 6,7's V projections feed only block 3's
                        # late segments; emitting them at block 3's start
                        # (not via the drip pool -- that races) shifts
                        # ~7us of PE work past the block-2 boundary flush
                        pool.extend(usA + usB)
                        deferred_v = vsA + vsB
                # weight the yproj fill toward late blocks, which are
                # exp-(ACT-)bound and need PE fill work: block 3 has ~30us
                # of PE slack, blocks 1-2 are mostly covered by proj units
                held = []
                if b == 2:
                    yp0 = make_yproj_units(0, o_tiles[0])
                    pool.extend(yp0[:8])
                    leftover_yp = yp0[8:]
                elif b == 3:
                    for u in deferred_v:
                        u()
                    yp = leftover_yp + make_yproj_units(1, o_tiles[1]) \
                        + make_yproj_units(2, o_tiles[2])
                    held = yp[-3:]  # cover the last pair's epilogue latency
                    pool.extend(yp[:-3])
                slots = 16 * (b + 1)
                per_slot = max(1, _math.ceil(len(pool) / slots))
                attention_block(b, q_tiles[b], o_tiles[b], pool, per_slot)
                while pool:  # flush before the next block depends on it
                    pool.popleft()()
            for u in held:  # PE work while the last pair's epilogue drains
                u()
            emit_pending()  # last pair of block 3
            for u in make_yproj_units(NIB - 1, o_tiles[NIB - 1],
                                      split_last=True):
                u()

    nc.finalize()
    return nc


def _prep_core_inputs(x, Wq, bq, Wk, bk, Wv, bv, Wp, core):
    import ml_dtypes
    BF = ml_dtypes.bfloat16
    b, g = core // G, core % G
    rows = slice(CG * g, CG * (g + 1))
    xT = np.ascontiguousarray(x[b].T).astype(BF)
    wqT = np.ascontiguousarray(Wq[rows, :].T).astype(BF)
    wkT = np.ascontiguousarray(Wk[rows, :].T).astype(BF)
    wvT = np.zeros((C, HPG * (D + 1)), dtype=np.float32)
    bv_aug = np.zeros(HPG * (D + 1), dtype=np.float32)
    for hl in range(HPG):
        cols = slice((D + 1) * hl, (D + 1) * hl + D)
        wvT[:, cols] = Wv[CG * g + D * hl: CG * g + D * (hl + 1), :].T
        bv_aug[cols] = bv[CG * g + D * hl: CG * g + D * (hl + 1)]
        bv_aug[(D + 1) * hl + D] = 1.0
    wpT = np.ascontiguousarray(Wp[:, rows].T).astype(BF)
    bqT = np.ascontiguousarray(bq[rows].reshape(CG // 128, 128).T)
    bkT = np.ascontiguousarray(bk[rows].reshape(CG // 128, 128).T)
    bvR = np.ascontiguousarray(np.tile(bv_aug[None, :], (128, 1)))
    tri = np.triu(np.ones((128, 128), dtype=np.float32)).astype(BF)
    ident = np.eye(128, dtype=np.float32).astype(BF)
    return {
        "xT": xT, "wq": wqT, "wk": wkT, "wv": wvT.astype(BF),
        "wp": wpT, "bq": bqT, "bk": bkT, "bv": bvR,
        "tri": tri, "ident": ident,
    }


def kernel(x, Wq, bq, Wk, bk, Wv, bv, Wp, bp, _trace=False):
    global LAST_RESULTS
    from concourse.bass_utils import run_bass_kernel_spmd

    x = np.asarray(x, dtype=np.float32)
    Wq, bq = np.asarray(Wq, np.float32), np.asarray(bq, np.float32)
    Wk, bk = np.asarray(Wk, np.float32), np.asarray(bk, np.float32)
    Wv, bv = np.asarray(Wv, np.float32), np.asarray(bv, np.float32)
    Wp, bp = np.asarray(Wp, np.float32), np.asarray(bp, np.float32)

    nc = _build_nc()
    in_maps = [
        _prep_core_inputs(x, Wq, bq, Wk, bk, Wv, bv, Wp, core)
        for core in range(B * G)
    ]
    res = run_bass_kernel_spmd(nc, in_maps, core_ids=list(range(8)), trace=_trace)
    LAST_RESULTS = res

    out = np.empty((B, T, C), dtype=np.float32)
    for b in range(B):
        out[b] = (np.asarray(res.results[G * b]["y"], dtype=np.float32)
                  + np.asarray(res.results[G * b + 1]["y"], dtype=np.float32)
                  + bp[None, :])
    return out
